# revision 13
# baseline (speedup 1.0000x reference)
"""Kalman CV filter (nn_KalmanCV) — Trainium2 Bass kernel, 8-core data parallel.

Math: the covariance P (and thus the Kalman gains K_t and the output
channels sx/sy/rho) is batch-independent — it depends only on the scalar
inputs. The whole per-batch computation therefore collapses to a linear
map over the 32 history scalars:

    out[l, b, ch<2] = sum_{t,ci} W[t*2+ci, l*2+ch] * hist[t, b, ci]
    out[l, b, ch>=2] = const[l, ch]          (sx, sy, rho)

Only the 50 data-dependent mu rows are computed on device; the 75
constant rows are filled host-side (they are input-data-independent,
like W itself). The matmul runs in fp16 (tolerance 2e-2, fp16 path
measures ~5e-4): 8x the fp32 PE rate and half the HBM bytes.

To double engine efficiency, two batch half-shards are packed into one
matmul with block-diagonal weights: lhsT (64, 100) with W in blocks
[0:32, 0:50] and [32:64, 50:100], rhs (64, n) holding half A in
partitions 0-31 and half B in 32-63. PSUM (100, n) then carries both
halves, so the PSUM->SBUF cast-copies (alternating Vector/Scalar
engines) run at 100/128 partition utilization instead of 50/128.
"""
import numpy as np

DT = 0.2
LEN_HIST = 16
LEN_PRED = 25
BATCH = 100000

N_CORES = 8
BS = 12800                  # padded batch per core
BS_REAL = BATCH // N_CORES  # 12500
HB = BS // 2                # 6400 columns per half-shard
K2 = 64                     # 2 stacked blocks of 32 history scalars
M2 = 100                    # 2 stacked blocks of 50 mu rows
NB = 512                    # matmul tile width (one PSUM bank)
# Input DMA chunks: small first chunk so the first matmul starts as
# early as possible; 512-aligned.
IN_CHUNKS = [(0, 512), (512, 1536), (2048, 2048), (4096, 2048), (6144, 256)]
# PSUM/copy/output chunks (gpsimd SWDGE ring): 2 matmuls share a 2-bank
# PSUM tile -> one cast-copy each; small last chunk shrinks the DMA tail.
OUT_CHUNKS = [(0, 1024), (1024, 1024), (2048, 1024), (3072, 1024),
              (4096, 1024), (5120, 1024), (6144, 256)]


def _build_wc(vsx, vsy, asx, asy, GR, coef_G, len_pred):
    """Collapse the filter to W (32, 5L) and constant vector cvec (5L,)."""
    L = int(len_pred)
    H = np.zeros((2, 4)); H[0, 0] = 1.0; H[1, 2] = 1.0
    F = np.eye(4); F[0, 1] = DT; F[2, 3] = DT
    G = np.array([DT * DT / 2, DT, DT * DT / 2, DT])
    Id = np.eye(4)

    ax2 = float(asx[0]) ** 2
    ay2 = float(asy[0]) ** 2
    mx = np.array([1.0, 1.0, 0.0, 0.0]); my = 1.0 - mx
    scale = (ax2 * np.outer(mx, mx) + ay2 * np.outer(my, my)
             + np.outer(mx, my) + np.outer(my, mx))
    g = G * np.tanh(np.asarray(coef_G, np.float64))
    Q = np.outer(g, g) * scale
    R = np.outer(np.asarray(GR, np.float64), np.asarray(GR, np.float64))

    D0 = np.array([[1.0, 0.0], [-1.0 / DT, 0.0], [0.0, 1.0], [0.0, -1.0 / DT]])
    D1 = np.array([[0.0, 0.0], [1.0 / DT, 0.0], [0.0, 0.0], [0.0, 1.0 / DT]])
    P = np.diag([R[0, 0], float(vsx[0]) ** 2, R[1, 1], float(vsy[0]) ** 2])

    C = np.zeros((LEN_HIST, 4, 2))
    C[0] = D0; C[1] = D1
    for t in range(1, LEN_HIST):
        P = F @ P @ F.T + Q
        S = H @ P @ H.T + R
        K = P @ H.T @ np.linalg.inv(S)
        A = (Id - K @ H) @ F
        C = np.einsum('ij,tjk->tik', A, C)
        C[t] += K
        ImKH = Id - K @ H
        P = ImKH @ P @ ImKH.T + K @ R @ K.T

    W_dev = np.zeros((2 * LEN_HIST, 5 * L))
    cvec = np.zeros(5 * L)
    M = np.eye(4)
    for l in range(L):
        M = F @ M
        P = F @ P @ F.T + Q
        HFl = H @ M
        Wl = np.einsum('ij,tjk->itk', HFl, C)   # (2, T, 2)
        for ch in range(2):
            W_dev[:, l * 5 + ch] = Wl[ch].reshape(-1)
        Pout = H @ P @ H.T
        sx = np.sqrt(Pout[0, 0]); sy = np.sqrt(Pout[1, 1])
        cvec[l * 5 + 2] = sx
        cvec[l * 5 + 3] = sy
        cvec[l * 5 + 4] = (Pout[0, 1] + Pout[1, 0]) / (2.0 * sx * sy)
    return W_dev.astype(np.float32), cvec.astype(np.float32)


_NC_CACHE = {}


def _build_bass():
    import concourse.bass as bass
    import concourse.bacc as bacc
    import concourse.tile as tile
    from concourse import mybir

    nc = bacc.Bacc("TRN2", target_bir_lowering=False, debug=False,
                   num_devices=N_CORES)
    x = nc.declare_dram_parameter("x", [K2, HB], mybir.dt.float16, isOutput=False)
    w = nc.declare_dram_parameter("w", [K2, M2], mybir.dt.float16, isOutput=False)
    out = nc.declare_dram_parameter("out", [M2, HB], mybir.dt.float16, isOutput=True)

    with tile.TileContext(nc) as tc:
        with tc.tile_pool(name="singles", bufs=1) as singles, \
             tc.tile_pool(name="xin", bufs=5) as xin_pool, \
             tc.tile_pool(name="ps", bufs=6, space="PSUM") as psum_pool, \
             tc.tile_pool(name="op", bufs=7) as out_pool:
            # Sync exits the framework preamble earliest: it kicks the
            # small first x chunk immediately, then the rest; w rides
            # the gpsimd (SWDGE) ring so it never delays an x transfer.
            w_tile = singles.tile([K2, M2], mybir.dt.float16)
            nc.gpsimd.dma_start(out=w_tile, in_=w[:, :])
            x_tiles = []
            for ioff, iw in IN_CHUNKS:
                t = xin_pool.tile([K2, iw], mybir.dt.float16)
                nc.sync.dma_start(out=t, in_=x[:, ioff:ioff + iw])
                x_tiles.append((ioff, iw, t))

            def x_slice(off, nw):
                for ioff, iw, t in x_tiles:
                    if ioff <= off and off + nw <= ioff + iw:
                        return t[:, off - ioff:off - ioff + nw]
                raise AssertionError("no input tile covers request")

            mm = 0
            for c, (goff, gw) in enumerate(OUT_CHUNKS):
                o_tile = out_pool.tile([M2, gw], mybir.dt.float16)
                off = 0
                while off < gw:
                    nw = min(NB, gw - off)
                    ps = psum_pool.tile([M2, nw], mybir.dt.float32)
                    nc.tensor.matmul(ps, w_tile, x_slice(goff + off, nw),
                                     start=True, stop=True)
                    dst = o_tile[:, off:off + nw]
                    # ScalarE is the cheaper copy engine per tile; it takes
                    # the odd tile of each pair so both engines run the two
                    # halves of a chunk concurrently.
                    if mm % 2 == 0:
                        nc.scalar.copy(out=dst, in_=ps)
                    else:
                        nc.vector.tensor_copy(out=dst, in_=ps)
                    mm += 1
                    off += nw
                # Spread output kicks across the three DMA-capable
                # engines; the final small chunk goes on scalar's
                # otherwise-empty ring so it drains with no backlog.
                if c == len(OUT_CHUNKS) - 1:
                    eng = nc.scalar
                else:
                    eng = nc.gpsimd if c % 2 == 0 else nc.sync
                eng.dma_start(out=out[:, goff:goff + gw], in_=o_tile)
    nc.compile()
    return nc


def _get_nc():
    if "nc" not in _NC_CACHE:
        _NC_CACHE["nc"] = _build_bass()
    return _NC_CACHE["nc"]


def _run_device(x_shards, W2, trace=False):
    from concourse.bass_utils import run_bass_kernel_spmd

    in_maps = [{"x": shard, "w": W2} for shard in x_shards]
    return run_bass_kernel_spmd(_get_nc(), in_maps, list(range(N_CORES)),
                                trace=trace)


def _make_shards(hist_T16):
    """hist_T16: (32, BATCH) f16 -> per-core (64, HB) stacked half-shards."""
    shards = []
    for c in range(N_CORES):
        xc = hist_T16[:, c * BS_REAL:(c + 1) * BS_REAL]  # (32, 12500)
        shard = np.zeros((K2, HB), np.float16)
        shard[0:32, :] = xc[:, :HB]
        shard[32:64, :BS_REAL - HB] = xc[:, HB:]
        shards.append(shard)
    return shards


def kernel(hist, velocity_std_x, velocity_std_y, acceleration_std_x,
           acceleration_std_y, GR, coef_G, len_pred):
    hist = np.asarray(hist, np.float32)
    L = int(len_pred)
    W, cvec = _build_wc(velocity_std_x, velocity_std_y, acceleration_std_x,
                        acceleration_std_y, GR, coef_G, L)
    T, B, _ = hist.shape
    hist_T = np.ascontiguousarray(hist.transpose(0, 2, 1)).reshape(2 * T, B)

    if L != LEN_PRED or B != BATCH or T != LEN_HIST:
        # shape surprise: fall back to exact host math
        out_flat = W.T @ hist_T + cvec[:, None]
        return np.ascontiguousarray(
            out_flat.reshape(L, 5, B).transpose(0, 2, 1)).astype(np.float32)

    # mu-only weight (32, 50), stacked block-diagonally to (64, 100)
    mu_cols = np.array([l * 5 + ch for l in range(LEN_PRED) for ch in range(2)])
    W_mu = W[:, mu_cols].astype(np.float16)
    W2 = np.zeros((K2, M2), np.float16)
    W2[0:32, 0:50] = W_mu
    W2[32:64, 50:100] = W_mu

    res = _run_device(_make_shards(hist_T.astype(np.float16)), W2)

    out = np.empty((LEN_PRED, B, 5), np.float32)
    consts = cvec.reshape(LEN_PRED, 5)[:, 2:5]           # (25, 3)
    out[:, :, 2:5] = consts[:, None, :]
    for c in range(N_CORES):
        oc = res.results[c]["out"]                       # (100, 6400) f16
        mu = np.concatenate(
            [oc[0:50, :], oc[50:100, :BS_REAL - HB]], axis=1)  # (50, 12500)
        out[:, c * BS_REAL:(c + 1) * BS_REAL, 0:2] = (
            mu.reshape(LEN_PRED, 2, BS_REAL).transpose(0, 2, 1))
    return out


# revision 16
# speedup vs baseline: 1.1592x; 1.1592x over previous
"""Kalman CV filter (nn_KalmanCV) — Trainium2 Bass kernel, 8-core data parallel.

Math: the covariance P (and thus the Kalman gains K_t and the output
channels sx/sy/rho) is batch-independent — it depends only on the scalar
inputs. The whole per-batch computation therefore collapses to a linear
map over the 32 history scalars:

    out[l, b, ch<2] = sum_{t,ci} W[t*2+ci, l*2+ch] * hist[t, b, ci]
    out[l, b, ch>=2] = const[l, ch]          (sx, sy, rho)

Only the 50 data-dependent mu rows are computed on device; the 75
constant rows are filled host-side (they are input-data-independent,
like W itself). The matmul runs in fp16 (tolerance 2e-2, fp16 path
measures ~5e-4): 8x the fp32 PE rate and half the HBM bytes.

To double engine efficiency, two batch half-shards are packed into one
matmul with block-diagonal weights: lhsT (64, 100) with W in blocks
[0:32, 0:50] and [32:64, 50:100], rhs (64, n) holding half A in
partitions 0-31 and half B in 32-63. PSUM (100, n) then carries both
halves, so the PSUM->SBUF cast-copies (alternating Vector/Scalar
engines) run at 100/128 partition utilization instead of 50/128.
"""
import numpy as np

DT = 0.2
LEN_HIST = 16
LEN_PRED = 25
BATCH = 100000

N_CORES = 8
BS = 12800                  # padded batch per core
BS_REAL = BATCH // N_CORES  # 12500
HB = BS // 2                # 6400 columns per half-shard
K2 = 64                     # 2 stacked blocks of 32 history scalars
M2 = 100                    # 2 stacked blocks of 50 mu rows
NB = 512                    # matmul tile width (one PSUM bank)
# Input DMA chunks: small first chunk so the first matmul starts as
# early as possible; fine-grained so each matmul's data arrives early.
IN_CHUNKS = [(0, 512), (512, 512), (1024, 1024), (2048, 1024),
             (3072, 1024), (4096, 1024), (5120, 1024), (6144, 256)]
# PSUM/copy/output chunks (gpsimd SWDGE ring): 2 matmuls share a 2-bank
# PSUM tile -> one cast-copy each; small last chunk shrinks the DMA tail.
OUT_CHUNKS = [(0, 1024), (1024, 1024), (2048, 1024), (3072, 1024),
              (4096, 1024), (5120, 1024), (6144, 256)]


def _build_wc(vsx, vsy, asx, asy, GR, coef_G, len_pred):
    """Collapse the filter to W (32, 5L) and constant vector cvec (5L,)."""
    L = int(len_pred)
    H = np.zeros((2, 4)); H[0, 0] = 1.0; H[1, 2] = 1.0
    F = np.eye(4); F[0, 1] = DT; F[2, 3] = DT
    G = np.array([DT * DT / 2, DT, DT * DT / 2, DT])
    Id = np.eye(4)

    ax2 = float(asx[0]) ** 2
    ay2 = float(asy[0]) ** 2
    mx = np.array([1.0, 1.0, 0.0, 0.0]); my = 1.0 - mx
    scale = (ax2 * np.outer(mx, mx) + ay2 * np.outer(my, my)
             + np.outer(mx, my) + np.outer(my, mx))
    g = G * np.tanh(np.asarray(coef_G, np.float64))
    Q = np.outer(g, g) * scale
    R = np.outer(np.asarray(GR, np.float64), np.asarray(GR, np.float64))

    D0 = np.array([[1.0, 0.0], [-1.0 / DT, 0.0], [0.0, 1.0], [0.0, -1.0 / DT]])
    D1 = np.array([[0.0, 0.0], [1.0 / DT, 0.0], [0.0, 0.0], [0.0, 1.0 / DT]])
    P = np.diag([R[0, 0], float(vsx[0]) ** 2, R[1, 1], float(vsy[0]) ** 2])

    C = np.zeros((LEN_HIST, 4, 2))
    C[0] = D0; C[1] = D1
    for t in range(1, LEN_HIST):
        P = F @ P @ F.T + Q
        S = H @ P @ H.T + R
        K = P @ H.T @ np.linalg.inv(S)
        A = (Id - K @ H) @ F
        C = np.einsum('ij,tjk->tik', A, C)
        C[t] += K
        ImKH = Id - K @ H
        P = ImKH @ P @ ImKH.T + K @ R @ K.T

    W_dev = np.zeros((2 * LEN_HIST, 5 * L))
    cvec = np.zeros(5 * L)
    M = np.eye(4)
    for l in range(L):
        M = F @ M
        P = F @ P @ F.T + Q
        HFl = H @ M
        Wl = np.einsum('ij,tjk->itk', HFl, C)   # (2, T, 2)
        for ch in range(2):
            W_dev[:, l * 5 + ch] = Wl[ch].reshape(-1)
        Pout = H @ P @ H.T
        sx = np.sqrt(Pout[0, 0]); sy = np.sqrt(Pout[1, 1])
        cvec[l * 5 + 2] = sx
        cvec[l * 5 + 3] = sy
        cvec[l * 5 + 4] = (Pout[0, 1] + Pout[1, 0]) / (2.0 * sx * sy)
    return W_dev.astype(np.float32), cvec.astype(np.float32)


_NC_CACHE = {}


def _build_bass():
    import concourse.bass as bass
    import concourse.bacc as bacc
    import concourse.tile as tile
    from concourse import mybir

    nc = bacc.Bacc("TRN2", target_bir_lowering=False, debug=False,
                   num_devices=N_CORES)
    x = nc.declare_dram_parameter("x", [K2, HB], mybir.dt.float16, isOutput=False)
    w = nc.declare_dram_parameter("w", [K2, M2], mybir.dt.float16, isOutput=False)
    out = nc.declare_dram_parameter("out", [M2, HB], mybir.dt.float16, isOutput=True)

    with tile.TileContext(nc) as tc:
        with tc.tile_pool(name="singles", bufs=1) as singles, \
             tc.tile_pool(name="xin", bufs=8) as xin_pool, \
             tc.tile_pool(name="ps", bufs=6, space="PSUM") as psum_pool, \
             tc.tile_pool(name="op", bufs=7) as out_pool:
            # Sync exits the framework preamble earliest: it kicks the
            # small first x chunk immediately, then the rest; w rides
            # the gpsimd (SWDGE) ring so it never delays an x transfer.
            w_tile = singles.tile([K2, M2], mybir.dt.float16)
            nc.gpsimd.dma_start(out=w_tile, in_=w[:, :])
            x_tiles = []
            for ioff, iw in IN_CHUNKS:
                t = xin_pool.tile([K2, iw], mybir.dt.float16)
                nc.sync.dma_start(out=t, in_=x[:, ioff:ioff + iw])
                x_tiles.append((ioff, iw, t))

            def x_slice(off, nw):
                for ioff, iw, t in x_tiles:
                    if ioff <= off and off + nw <= ioff + iw:
                        return t[:, off - ioff:off - ioff + nw]
                raise AssertionError("no input tile covers request")

            mm = 0
            for c, (goff, gw) in enumerate(OUT_CHUNKS):
                o_tile = out_pool.tile([M2, gw], mybir.dt.float16)
                off = 0
                while off < gw:
                    nw = min(NB, gw - off)
                    ps = psum_pool.tile([M2, nw], mybir.dt.float32)
                    nc.tensor.matmul(ps, w_tile, x_slice(goff + off, nw),
                                     start=True, stop=True)
                    dst = o_tile[:, off:off + nw]
                    # ScalarE is the cheaper copy engine per tile; it takes
                    # the odd tile of each pair so both engines run the two
                    # halves of a chunk concurrently.
                    if mm % 2 == 0:
                        nc.scalar.copy(out=dst, in_=ps)
                    else:
                        nc.vector.tensor_copy(out=dst, in_=ps)
                    mm += 1
                    off += nw
                # Spread output kicks across the three DMA-capable
                # engines. Scalar only takes the last two chunks, after
                # its copy stream is done, so kicks never stall copies;
                # its ring is otherwise empty so the tail drains fast.
                eng = {0: nc.gpsimd, 1: nc.sync, 2: nc.gpsimd, 3: nc.sync,
                       4: nc.gpsimd, 5: nc.scalar, 6: nc.scalar}[c]
                eng.dma_start(out=out[:, goff:goff + gw], in_=o_tile)
    nc.compile()
    return nc


def _get_nc():
    if "nc" not in _NC_CACHE:
        _NC_CACHE["nc"] = _build_bass()
    return _NC_CACHE["nc"]


def _run_device(x_shards, W2, trace=False):
    from concourse.bass_utils import run_bass_kernel_spmd

    in_maps = [{"x": shard, "w": W2} for shard in x_shards]
    return run_bass_kernel_spmd(_get_nc(), in_maps, list(range(N_CORES)),
                                trace=trace)


def _make_shards(hist_T16):
    """hist_T16: (32, BATCH) f16 -> per-core (64, HB) stacked half-shards."""
    shards = []
    for c in range(N_CORES):
        xc = hist_T16[:, c * BS_REAL:(c + 1) * BS_REAL]  # (32, 12500)
        shard = np.zeros((K2, HB), np.float16)
        shard[0:32, :] = xc[:, :HB]
        shard[32:64, :BS_REAL - HB] = xc[:, HB:]
        shards.append(shard)
    return shards


def kernel(hist, velocity_std_x, velocity_std_y, acceleration_std_x,
           acceleration_std_y, GR, coef_G, len_pred):
    hist = np.asarray(hist, np.float32)
    L = int(len_pred)
    W, cvec = _build_wc(velocity_std_x, velocity_std_y, acceleration_std_x,
                        acceleration_std_y, GR, coef_G, L)
    T, B, _ = hist.shape
    hist_T = np.ascontiguousarray(hist.transpose(0, 2, 1)).reshape(2 * T, B)

    if L != LEN_PRED or B != BATCH or T != LEN_HIST:
        # shape surprise: fall back to exact host math
        out_flat = W.T @ hist_T + cvec[:, None]
        return np.ascontiguousarray(
            out_flat.reshape(L, 5, B).transpose(0, 2, 1)).astype(np.float32)

    # mu-only weight (32, 50), stacked block-diagonally to (64, 100)
    mu_cols = np.array([l * 5 + ch for l in range(LEN_PRED) for ch in range(2)])
    W_mu = W[:, mu_cols].astype(np.float16)
    W2 = np.zeros((K2, M2), np.float16)
    W2[0:32, 0:50] = W_mu
    W2[32:64, 50:100] = W_mu

    res = _run_device(_make_shards(hist_T.astype(np.float16)), W2)

    out = np.empty((LEN_PRED, B, 5), np.float32)
    consts = cvec.reshape(LEN_PRED, 5)[:, 2:5]           # (25, 3)
    out[:, :, 2:5] = consts[:, None, :]
    for c in range(N_CORES):
        oc = res.results[c]["out"]                       # (100, 6400) f16
        mu = np.concatenate(
            [oc[0:50, :], oc[50:100, :BS_REAL - HB]], axis=1)  # (50, 12500)
        out[:, c * BS_REAL:(c + 1) * BS_REAL, 0:2] = (
            mu.reshape(LEN_PRED, 2, BS_REAL).transpose(0, 2, 1))
    return out
